# revision 16
# baseline (speedup 1.0000x reference)
"""Multi-head attention (B=4, T=2048, C=1024, 16 heads, no mask) on 8 TRN2 cores.

Sharding: pure query-sharding — core c handles batch b=c//2, query-half
ih=c%2 (1024 query rows). Each core computes K/V for its whole batch
(duplicated across the 2 cores sharing a batch) and its own Q rows, runs
full attention + output projection locally. Zero on-chip collectives.

Host-side prep (part of sharding): x[b] is passed transposed (xT[c,t]) with
the core's query rows rotated to the front; weights are passed transposed
(W.T, [c_in, c_out]). Attention is permutation-invariant over keys, so the
rotated key order is harmless.

Device dataflow per core (all matmuls float32r: FP22 multiply / FP32 acc,
full bf16-rate when moving-dim>=256):
  P1: kT[o,t] = WkT.T @ xT          (stationary WkT tiles, moving xT)
      qT[o,i] likewise (first 1024 cols of xT)
      v[t,o]  = xT_tile.T @ WvT     -> DRAM scratch (SBUF pressure)
  P2: per (i-chunk 512, head-pair p, j-tile g):
      scoresT[j,i] = kT_h.T-slice @ qT_h  (two heads row-packed, concurrent)
      probsT = exp(scoresT/8)   (ScalarE, scale folded into ACT)
      yT[o,i] += [v_h | ones].T @ probsT  (ones col -> row 64 = softmax denom)
      yT /= denom ; out[i,:] = sum_p yT[p].T-slice @ WoT[p]
"""

import os
import numpy as np

B, T, C = 4, 2048, 1024
NH, HS = 16, 64
N_CORES = 8

_CACHE = {}
LAST_RESULTS = {}


def _build_nc(debug_taps=False):
    import concourse.bass as bass
    import concourse.mybir as mybir
    import concourse.tile as tile
    from concourse import bacc

    F32 = mybir.dt.float32
    F32R = mybir.dt.float32r
    AF = mybir.ActivationFunctionType
    ALU = mybir.AluOpType

    nc = bacc.Bacc("TRN2", target_bir_lowering=False, debug=False, num_devices=N_CORES)

    xT = nc.dram_tensor("xT", [C, T], F32R, kind="ExternalInput").ap()
    wkT = nc.dram_tensor("wkT", [C, C], F32R, kind="ExternalInput").ap()
    wqT = nc.dram_tensor("wqT", [C, C], F32R, kind="ExternalInput").ap()
    wvT = nc.dram_tensor("wvT", [C, C], F32R, kind="ExternalInput").ap()
    woT = nc.dram_tensor("woT", [C, C], F32R, kind="ExternalInput").ap()
    out = nc.dram_tensor("out", [1024, C], F32, kind="ExternalOutput").ap()
    v_scr = nc.dram_tensor("v_scr", [T, C], F32R,
                           kind="ExternalOutput" if debug_taps else "Internal").ap()
    if debug_taps:
        dbg_k = nc.dram_tensor("dbg_k", [128, T], F32, kind="ExternalOutput").ap()
        dbg_q = nc.dram_tensor("dbg_q", [128, 1024], F32, kind="ExternalOutput").ap()
        dbg_y = nc.dram_tensor("dbg_y", [128, 512], F32, kind="ExternalOutput").ap()
        dbg_rec = nc.dram_tensor("dbg_rec", [128, 512], F32, kind="ExternalOutput").ap()
        dbg_pr = nc.dram_tensor("dbg_pr", [128, 1024], F32, kind="ExternalOutput").ap()
        dbg_vA = nc.dram_tensor("dbg_vA", [128, 16, 65], F32, kind="ExternalOutput").ap()
        dbg_yraw = nc.dram_tensor("dbg_yraw", [128, 1024], F32, kind="ExternalOutput").ap()
        dbg_recraw = nc.dram_tensor("dbg_recraw", [128, 512], F32, kind="ExternalOutput").ap()
        dbg_vB = nc.dram_tensor("dbg_vB", [128, 16, 65], F32, kind="ExternalOutput").ap()

    with tile.TileContext(nc) as tc:
        with tc.tile_pool(name="res", bufs=1) as res:
            kT = [res.tile([128, T], F32R, tag=f"kT{p}", name=f"kT{p}") for p in range(8)]
            qT = [res.tile([128, 1024], F32R, tag=f"qT{p}", name=f"qT{p}") for p in range(8)]

            # ---------------- Phase 1: projections ----------------
            with tc.tile_pool(name="p1", bufs=1) as p1:
                xTs = [p1.tile([128, T], F32R, tag=f"xT{c}", name=f"xT{c}") for c in range(8)]
                for c in range(8):
                    nc.sync.dma_start(xTs[c][:], xT[c * 128:(c + 1) * 128, :])

                # v = x @ Wv.T, layout [t, o]; two o-half passes to keep SBUF low
                with tc.tile_pool(name="vps", bufs=4, space="PSUM") as vps:
                    for och in range(2):
                        wvh = [p1.tile([128, 512], F32R, tag=f"wv{c}", name=f"wv{och}_{c}")
                               for c in range(8)]
                        for c in range(8):
                            nc.sync.dma_start(wvh[c][:], wvT[c * 128:(c + 1) * 128,
                                                             och * 512:(och + 1) * 512])
                        for tt in range(16):
                            ps = vps.tile([128, 512], F32, tag="vproj", name=f"vps{och}_{tt}")
                            for c in range(8):
                                nc.tensor.matmul(
                                    ps[:, :],
                                    xTs[c][:, tt * 128:(tt + 1) * 128],
                                    wvh[c][:, :],
                                    start=(c == 0), stop=(c == 7),
                                )
                            vb = p1.tile([128, 512], F32R, tag="vev", bufs=3, name=f"vev{och}_{tt}")
                            nc.scalar.copy(vb[:], ps[:])
                            nc.sync.dma_start(
                                v_scr[tt * 128:(tt + 1) * 128, och * 512:(och + 1) * 512], vb[:])

                # kT = (Wk x.T) in [o, t] layout; E/O psum alternation
                with tc.tile_pool(name="kps", bufs=1, space="PSUM") as kps:
                    for p in range(8):
                        ps = kps.tile([128, T], F32, tag=("kE" if p % 2 == 0 else "kO"),
                                      name=f"kps{p}")
                        for c in range(8):
                            w = p1.tile([128, 128], F32R, tag="wtile", bufs=4, name=f"wk{p}_{c}")
                            nc.sync.dma_start(w[:], wkT[c * 128:(c + 1) * 128,
                                                        p * 128:(p + 1) * 128])
                            for t4 in range(4):
                                nc.tensor.matmul(
                                    ps[:, t4 * 512:(t4 + 1) * 512],
                                    w[:],
                                    xTs[c][:, t4 * 512:(t4 + 1) * 512],
                                    start=(c == 0), stop=(c == 7),
                                )
                        nc.scalar.copy(kT[p][:], ps[:])
                        if debug_taps and p == 0:
                            nc.sync.dma_start(dbg_k[:], kT[p][:].bitcast(F32))

                # qT: queries are local t 0..1023
                with tc.tile_pool(name="qps", bufs=2, space="PSUM") as qps:
                    for p in range(8):
                        ps = qps.tile([128, 1024], F32, tag="qproj", name=f"qps{p}")
                        for c in range(8):
                            w = p1.tile([128, 128], F32R, tag="wtile", bufs=4, name=f"wq{p}_{c}")
                            nc.sync.dma_start(w[:], wqT[c * 128:(c + 1) * 128,
                                                        p * 128:(p + 1) * 128])
                            for t2 in range(2):
                                nc.tensor.matmul(
                                    ps[:, t2 * 512:(t2 + 1) * 512],
                                    w[:],
                                    xTs[c][:, t2 * 512:(t2 + 1) * 512],
                                    start=(c == 0), stop=(c == 7),
                                )
                        nc.scalar.copy(qT[p][:], ps[:])
                        if debug_taps and p == 0:
                            nc.sync.dma_start(dbg_q[:], qT[p][:].bitcast(F32))

            # ---------------- Phase 2: attention + out proj ----------------
            with tc.tile_pool(name="p2", bufs=1) as p2, \
                 tc.tile_pool(name="p2ps", bufs=1, space="PSUM") as p2ps:
                wos = [p2.tile([128, C], F32R, tag=f"wo{p}", name=f"wo{p}") for p in range(8)]
                for p in range(8):
                    nc.sync.dma_start(wos[p][:], woT[p * 128:(p + 1) * 128, :])

                v_r = v_scr.rearrange("(a p) c -> p a c", p=128)

                for ic in range(2):
                    yTs = [p2.tile([128, 512], F32R, tag=f"yT{p}", name=f"yT{ic}_{p}")
                           for p in range(8)]
                    for p in range(8):
                        # v for heads 2p (A) and 2p+1 (B), with ones col at 64
                        vA = p2.tile([128, 16, 65], F32R, tag="vaugA", bufs=2,
                                     name=f"vA{ic}_{p}")
                        vB = p2.tile([128, 16, 65], F32R, tag="vaugB", bufs=2,
                                     name=f"vB{ic}_{p}")
                        hA, hB = 2 * p, 2 * p + 1
                        nc.sync.dma_start(vA[:, :, 0:64],
                                          v_r[:, :, hA * 64:(hA + 1) * 64])
                        nc.sync.dma_start(vB[:, :, 0:64],
                                          v_r[:, :, hB * 64:(hB + 1) * 64])
                        nc.vector.memset(vA[:, :, 64:65].bitcast(F32), 1.0)
                        nc.vector.memset(vB[:, :, 64:65].bitcast(F32), 1.0)
                        if debug_taps and ic == 0 and p == 0:
                            nc.sync.dma_start(dbg_vA[:], vA[:].bitcast(F32))
                            nc.sync.dma_start(dbg_vB[:], vB[:].bitcast(F32))

                        yA = p2ps.tile([128, 512], F32, tag="yA", name=f"yA{ic}_{p}")
                        yB = p2ps.tile([128, 512], F32, tag="yB", name=f"yB{ic}_{p}")
                        for g in range(16):
                            stag = "sE" if g % 2 == 0 else "sO"
                            s = p2ps.tile([128, 1024], F32, tag=stag,
                                          name=f"s{ic}_{p}_{g}")
                            nc.tensor.matmul(
                                s[:, 0:512],
                                kT[p][0:64, g * 128:(g + 1) * 128],
                                qT[p][0:64, ic * 512:(ic + 1) * 512],
                                start=True, stop=True)
                            nc.tensor.matmul(
                                s[:, 512:1024],
                                kT[p][64:128, g * 128:(g + 1) * 128],
                                qT[p][64:128, ic * 512:(ic + 1) * 512],
                                start=True, stop=True)
                            prA = p2.tile([128, 512], F32R, tag="prA" + stag,
                                          name=f"prA{ic}_{p}_{g}")
                            prB = p2.tile([128, 512], F32R, tag="prB" + stag,
                                          name=f"prB{ic}_{p}_{g}")
                            nc.scalar.activation(prA[:], s[:, 0:512], AF.Exp, scale=0.125)
                            nc.scalar.activation(prB[:], s[:, 512:1024], AF.Exp, scale=0.125)
                            if debug_taps and ic == 0 and p == 0 and g == 0:
                                nc.sync.dma_start(dbg_pr[:, 0:512], prA[:].bitcast(F32))
                                nc.sync.dma_start(dbg_pr[:, 512:1024], prB[:].bitcast(F32))
                            nc.tensor.matmul(yB[0:65, :], vB[:, g, :],
                                             prB[:],
                                             start=(g == 0), stop=(g == 15))
                            nc.tensor.matmul(yA[0:65, :], vA[:, g, :],
                                             prA[:],
                                             start=(g == 0), stop=(g == 15))

                        if debug_taps and ic == 0 and p == 0:
                            yrawt = p2.tile([128, 1024], F32, tag="yrawt", name="yrawt")
                            nc.scalar.copy(yrawt[:, 0:512], yA[:])
                            nc.scalar.copy(yrawt[:, 512:1024], yB[:])
                            nc.sync.dma_start(dbg_yraw[:], yrawt[:])
                        rec = p2.tile([128, 512], F32, tag="rec", name=f"rec{ic}_{p}")
                        nc.vector.reciprocal(rec[0:1, :], yA[64:65, :])
                        nc.vector.reciprocal(rec[64:65, :], yB[64:65, :])
                        if debug_taps and ic == 0 and p == 0:
                            nc.sync.dma_start(dbg_recraw[:], rec[:])
                        bc = p2.tile([128, 512], F32, tag="bc", name=f"bc{ic}_{p}")
                        nc.sync.dma_start(
                            bc[0:64, :],
                            rec[0:1, :].unsqueeze(1).to_broadcast((1, 64, 512)))
                        nc.sync.dma_start(
                            bc[64:128, :],
                            rec[64:65, :].unsqueeze(1).to_broadcast((1, 64, 512)))
                        nc.vector.tensor_tensor(out=yTs[p][0:64, :], in0=yA[0:64, :],
                                                in1=bc[0:64, :], op=ALU.mult)
                        nc.vector.tensor_tensor(out=yTs[p][64:128, :], in0=yB[0:64, :],
                                                in1=bc[64:128, :], op=ALU.mult)
                        if debug_taps and ic == 0 and p == 0:
                            nc.sync.dma_start(dbg_y[:], yTs[p][:].bitcast(F32))
                            nc.sync.dma_start(dbg_rec[:], bc[:])

                    # final projection for this i-chunk
                    for it in range(4):
                        fps = p2ps.tile([128, 1024], F32, tag="fin", name=f"fin{ic}_{it}")
                        for p in range(8):
                            for n2 in range(2):
                                nc.tensor.matmul(
                                    fps[:, n2 * 512:(n2 + 1) * 512],
                                    yTs[p][:, it * 128:(it + 1) * 128],
                                    wos[p][:, n2 * 512:(n2 + 1) * 512],
                                    start=(p == 0), stop=(p == 7))
                        ob = p2.tile([128, 1024], F32, tag="ob", bufs=2,
                                     name=f"ob{ic}_{it}")
                        nc.scalar.copy(ob[:], fps[:])
                        nc.sync.dma_start(
                            out[ic * 512 + it * 128: ic * 512 + (it + 1) * 128, :],
                            ob[:])

    nc.compile()
    return nc


def _get_nc():
    if "nc" not in _CACHE:
        _CACHE["nc"] = _build_nc()
    return _CACHE["nc"]


def _make_in_maps(x, Wk, Wq, Wv, Wo):
    wkT = np.ascontiguousarray(Wk.T.astype(np.float32))
    wqT = np.ascontiguousarray(Wq.T.astype(np.float32))
    wvT = np.ascontiguousarray(Wv.T.astype(np.float32))
    woT = np.ascontiguousarray(Wo.T.astype(np.float32))
    in_maps = []
    for core in range(N_CORES):
        b, ih = core // 2, core % 2
        xb = np.asarray(x[b], dtype=np.float32)
        if ih == 0:
            xloc = xb
        else:
            xloc = np.concatenate([xb[1024:], xb[:1024]], axis=0)
        in_maps.append({
            "xT": np.ascontiguousarray(xloc.T),
            "wkT": wkT, "wqT": wqT, "wvT": wvT, "woT": woT,
        })
    return in_maps


def _install_ntff_hook_shim():
    """The agent image's antenv lacks axon_hooks; recreate it so
    run_bass_kernel_spmd(trace=True) can capture NTFF profiles."""
    import sys, types
    try:
        from antenv.axon_hooks import get_axon_ntff_profile_hook  # noqa
        return True
    except ImportError:
        pass
    try:
        sys.path.insert(0, "/root/.axon_site")
        from trn_agent_boot.trn_boot import _ntff_profile_via_ctypes
        hook = _ntff_profile_via_ctypes("/opt/axon/libaxon_pjrt.so")
        if hook is None:
            return False
        mod = types.ModuleType("antenv.axon_hooks")
        mod._hook = hook
        mod.get_axon_ntff_profile_hook = lambda: mod._hook
        mod.set_axon_ntff_profile_hook = lambda h: setattr(mod, "_hook", h)
        sys.modules["antenv.axon_hooks"] = mod
        import antenv
        antenv.axon_hooks = mod
        return True
    except Exception:
        return False


def kernel(x, Wk, Wq, Wv, Wo):
    from concourse.bass_utils import run_bass_kernel_spmd

    nc = _get_nc()
    in_maps = _make_in_maps(x, Wk, Wq, Wv, Wo)
    trace = bool(int(os.environ.get("ATT_TRACE", "0")))
    if trace and not _install_ntff_hook_shim():
        trace = False
    res = run_bass_kernel_spmd(nc, in_maps, core_ids=list(range(N_CORES)),
                               trace=trace)
    LAST_RESULTS["exec_time_ns"] = res.exec_time_ns
    LAST_RESULTS["res"] = res
    full = np.empty((B, T, C), dtype=np.float32)
    for core in range(N_CORES):
        b, ih = core // 2, core % 2
        full[b, ih * 1024:(ih + 1) * 1024] = res.results[core]["out"]
    return full
